# revision 65
# baseline (speedup 1.0000x reference)
# Trainium2 Bass kernel for nn_Attention_48052094107920 (sparse_attention).
# See build_nc3 docstring comments below for the design.
# v3: batch-pair x query-half sharding with host-folded mask weights.
#
# Core c = (batch-pair p = c//2, q-half s = c%2). Each core processes its 2
# batches for ALL 12 local heads over a 289-column query slice (s=0 covers
# q 0:289, s=1 covers q 288:577; the host drops the overlap column). Outputs
# are full projections (transposed layout); the host transposes, adds
# proj_b, and concatenates -- no cross-core reduction.
#
# Key design points vs v2:
#   - mw[g,h] = sum_m mask_proj[m, g*12+h] * masks_m is computed on the HOST
#     (batch-independent weight folding) and streamed per-head from DRAM
#     (13.3MB/core, overlapped with the head loop under the DMA-device
#     budget). Kills the on-chip DVE precompute entirely.
#   - mix at_h = sum_g S_g * mw[g,h]: products for g=1,2 on the Pool
#     (gpsimd) engine (otherwise idle, tensor ops at 1.2GHz), the g=0
#     product and both adds on DVE (fp16 2x mode).
#   - exp: ONE activation per (b,h): padded k rows have S=0 (zero-padded x)
#     and mw=0 (host zeros) -> at=0 -> e=1, and vhat rows there are 0
#     (including the ones-column that generates Z), so pads contribute
#     nothing to p@v or Z. No pad-bias pass.
#   - v projection contracts only the real 768 channels (6 steps); the
#     per-head ones-columns that generate Z during p@v come from a tiny
#     host-provided token-validity mask copied in by the Pool engine.
#   - output projection emitted transposed ([c-chunk, q]): the contraction
#     runs as 12 64-deep steps costing q-width per step, and the lhsT is the
#     proj weight directly; host undoes the transpose.
#   - head loop interleaves the two batches with batch 0 running 3 heads
#     ahead, so batch 1's phase A overlaps batch 0's first heads and every
#     in-order engine queue stays busy; mw tiles rotate through 4 buffers.
#   - softmax normalization per 4-head group: in-place reciprocal on the Z
#     row of the o/Z staging tile, DMA partition-broadcast, one multiply.

import numpy as np

import concourse.bass as bass
import concourse.bacc as bacc_mod
import concourse.mybir as mybir
import concourse.tile as tile
from concourse import bass_utils

BF = mybir.dt.float16
F32 = mybir.dt.float32
AF = mybir.ActivationFunctionType
OP = mybir.AluOpType

B, N, C = 8, 577, 768
GH, LH, ML, HD = 3, 12, 3, 64
SCALE = HD ** -0.5
NP = 640          # padded tokens (5 * 128)
NJ = 5            # k chunks of 128
KQ = 6            # contraction chunks (768 channels)
QW = 289          # query-half width (s=0: 0:289, s=1: 288:577)
VW = HD + 1       # 65: per-head v block [v | ones]


def build_nc3():
    nc = bacc_mod.Bacc("TRN2", target_bir_lowering=False, debug=False, num_devices=8)

    xt = nc.dram_tensor("xt", [128, 2, KQ, NP], BF, kind="ExternalInput")
    xq = nc.dram_tensor("xq", [128, 2, KQ, QW], BF, kind="ExternalInput")
    wq = nc.dram_tensor("wq", [128, KQ, GH * HD], BF, kind="ExternalInput")
    wk = nc.dram_tensor("wk", [128, KQ, GH * HD], BF, kind="ExternalInput")
    wv = nc.dram_tensor("wv", [128, KQ, LH * VW], BF, kind="ExternalInput")
    vm = nc.dram_tensor("vm", [128, NJ, LH], BF, kind="ExternalInput")
    pw = nc.dram_tensor("pw", [128, 6, C], BF, kind="ExternalInput")
    mw = nc.dram_tensor("mw", [128, LH, GH, NJ, QW], BF, kind="ExternalInput")
    out = nc.dram_tensor("ot", [2, 6, 128, QW], BF, kind="ExternalOutput")

    with tile.TileContext(nc) as tc, \
         tc.tile_pool(name="const", bufs=1) as cpool, \
         tc.tile_pool(name="mwst", bufs=5) as mpool, \
         tc.tile_pool(name="work", bufs=2) as wpool, \
         tc.tile_pool(name="atp", bufs=4) as atpool, \
         tc.tile_pool(name="ttp", bufs=3) as ttpool, \
         tc.tile_pool(name="ep", bufs=6) as epool, \
         tc.tile_pool(name="psA", bufs=2, space="PSUM") as ppA, \
         tc.tile_pool(name="psS", bufs=3, space="PSUM") as ppS, \
         tc.tile_pool(name="psO", bufs=2, space="PSUM") as ppO, \
         tc.tile_pool(name="psZ", bufs=1, space="PSUM") as ppZ:

        # ---- input loads, ordered so compute starts ASAP ----
        wq_s = cpool.tile([128, KQ, GH * HD], BF, tag="wq")
        nc.sync.dma_start(wq_s[:], wq.ap())
        xq_s = cpool.tile([128, 2, KQ, QW], BF, tag="xq")
        nc.sync.dma_start(xq_s[:, 0], xq.ap()[:, 0])
        wk_s = cpool.tile([128, KQ, GH * HD], BF, tag="wk")
        nc.sync.dma_start(wk_s[:], wk.ap())
        xt_s = cpool.tile([128, 2, KQ, NP], BF, tag="xt")
        nc.sync.dma_start(xt_s[:, 0], xt.ap()[:, 0])
        def mw_load(h):
            t = mpool.tile([128, GH, NJ, QW], BF, tag="mwh", name=f"mw{h}")
            nc.sync.dma_start(t[:], mw.ap()[:, h])
            return t

        mwq = {0: mw_load(0)}
        wv_s = cpool.tile([128, KQ, LH * VW], BF, tag="wv")
        nc.sync.dma_start(wv_s[:], wv.ap())
        vm_s = cpool.tile([128, NJ, LH], BF, tag="vm")
        nc.sync.dma_start(vm_s[:], vm.ap())
        mwq[1] = mw_load(1)
        nc.sync.dma_start(xq_s[:, 1], xq.ap()[:, 1])
        nc.sync.dma_start(xt_s[:, 1], xt.ap()[:, 1])
        mwq[2] = mw_load(2)
        mwq[3] = mw_load(3)
        mwq[4] = mw_load(4)

        # persistent per-batch state
        qT01 = [cpool.tile([128, QW], BF, tag=f"q01_{b}", name=f"q01_{b}") for b in range(2)]
        qT2 = [cpool.tile([64, QW], BF, tag=f"q2_{b}", name=f"q2_{b}") for b in range(2)]
        kT01 = [cpool.tile([128, NP], BF, tag=f"k01_{b}", name=f"k01_{b}") for b in range(2)]
        kT2 = [cpool.tile([64, NP], BF, tag=f"k2_{b}", name=f"k2_{b}") for b in range(2)]
        vt = [cpool.tile([128, NJ, LH * VW], BF, tag=f"vt_{b}", name=f"vt_{b}") for b in range(2)]
        ssb = [cpool.tile([128, GH, NJ, QW], BF, tag=f"ssb_{b}", name=f"ssb_{b}") for b in range(2)]
        povs = [cpool.tile([VW, LH, QW], BF, tag=f"pov_{b}", name=f"pov_{b}") for b in range(2)]
        on = [[cpool.tile([128, 2, QW], BF, tag=f"on_{b}_{g}", name=f"on_{b}_{g}")
               for g in range(3)] for b in range(2)]
        pw_box = [None]
        ones_t = cpool.tile([VW, 64], BF, tag="ones")
        nc.vector.memset(ones_t[:], 1.0)

        def qk_proj(b):
            cp = nc.scalar.copy if b == 0 else nc.vector.tensor_copy
            # q projection (289 cols), channels on partitions
            for msl, mp, dst in ((slice(0, 128), 128, qT01[b]),
                                 (slice(128, 192), 64, qT2[b])):
                ps = ppA.tile([128, 512], F32, tag="bigA", name="psA")[:mp, :QW]
                for o in range(KQ):
                    nc.tensor.matmul(ps, wq_s[:, o, msl], xq_s[:, b, o, :],
                                     start=(o == 0), stop=(o == KQ - 1))
                cp(dst[:mp, :], ps)
            # k projection (full 640; padded tokens project to 0)
            for msl, mp, dst in ((slice(0, 128), 128, kT01[b]),
                                 (slice(128, 192), 64, kT2[b])):
                for n0, n1 in ((0, 512), (512, NP)):
                    ps = ppA.tile([128, 512], F32, tag="bigA", name="psA")[:mp, : n1 - n0]
                    for o in range(KQ):
                        nc.tensor.matmul(ps, wk_s[:, o, msl], xt_s[:, b, o, n0:n1],
                                         start=(o == 0), stop=(o == KQ - 1))
                    cp(dst[:mp, n0:n1], ps)

        def v_proj(b, half):
            # v-hat projection for heads [6*half, 6*half+6): tokens on
            # partitions, interleaved [v_h | 0] blocks; the zero
            # ones-columns are then filled from vm
            cp = nc.scalar.copy
            n0, n1 = half * 6 * VW, (half + 1) * 6 * VW
            for kc in range(NJ):
                ps = ppA.tile([128, 512], F32, tag="bigA", name="psA")[:, : n1 - n0]
                for o in range(KQ):
                    nc.tensor.matmul(ps, xt_s[:, b, o, kc * 128:(kc + 1) * 128],
                                     wv_s[:, o, n0:n1],
                                     start=(o == 0), stop=(o == KQ - 1))
                cp(vt[b][:, kc, n0:n1], ps)
                nc.gpsimd.tensor_copy(vt[b][:, kc, n0 + HD:n1:VW],
                                      vm_s[:, kc, half * 6:half * 6 + 6])

        def qg(b, g):
            return (qT01[b][0:64], qT01[b][64:128], qT2[b][0:64])[g]

        def kg(b, g):
            return (kT01[b][0:64], kT01[b][64:128], kT2[b][0:64])[g]

        def scores(b):
            for g in range(GH):
                for j in range(NJ):
                    ps = ppS.tile([128, QW], F32, tag="s", name="psS")
                    nc.tensor.matmul(ps, kg(b, g)[:, j * 128:(j + 1) * 128],
                                     qg(b, g), start=True, stop=True)
                    if b == 0:
                        nc.vector.tensor_copy(ssb[b][:, g, j], ps)
                    else:
                        nc.scalar.copy(ssb[b][:, g, j], ps)

        def head_mix(b, h, mwt):
            at = atpool.tile([128, NJ, QW], BF, tag="at")
            tb = ttpool.tile([128, NJ, QW], BF, tag="tb")
            tt = ttpool.tile([128, NJ, QW], BF, tag="tt")
            nc.gpsimd.tensor_mul(tb[:], ssb[b][:, 1], mwt[:, 1])
            nc.gpsimd.tensor_mul(tt[:], ssb[b][:, 2], mwt[:, 2])
            nc.vector.tensor_mul(at[:], ssb[b][:, 0], mwt[:, 0])
            nc.vector.tensor_add(at[:], at[:], tb[:])
            nc.vector.tensor_add(at[:], at[:], tt[:])
            e = epool.tile([128, NJ, QW], BF, tag="e")
            nc.scalar.activation(e[:], at[:], AF.Exp)
            return e

        def head_pav(b, h, e):
            pov = ppO.tile([VW, QW], F32, tag="ov", name="psO")
            for j in range(NJ):
                nc.tensor.matmul(pov, vt[b][:, j, h * VW:(h + 1) * VW], e[:, j, :],
                                 start=(j == 0), stop=(j == NJ - 1))
            if b == 1 and h == LH - 2:
                nc.vector.tensor_copy(povs[b][:, h], pov)
            else:
                nc.scalar.copy(povs[b][:, h], pov)

        def head(b, h, mwt):
            head_pav(b, h, head_mix(b, h, mwt))

        def tail4(b, h0):
            hs = slice(h0, h0 + 4)
            g = h0 // 4
            if (b, h0) == (0, 8):
                # tail-critical group: in-place reciprocal, lowest latency
                with nc.allow_low_precision(reason="Z in f16 range; 2e-2 tol"):
                    nc.vector.reciprocal(povs[b][64:65, hs], povs[b][64:65, hs])
            else:
                # slack groups: spread the 4 Z rows over partitions 0:4 so
                # the reciprocal costs free-size 289 instead of 1156 on the
                # binding DVE, then scatter back
                zg = cpool.tile([4, QW], BF, tag=f"zg{b}{g}", name=f"zg{b}{g}")
                nc.sync.dma_start(zg[:], povs[b][64:65, hs, :])
                with nc.allow_low_precision(reason="Z in f16 range; 2e-2 tol"):
                    nc.vector.reciprocal(zg[:], zg[:])
                nc.sync.dma_start(povs[b][64:65, hs, :], zg[:])
            zrep = cpool.tile([64, 4, QW], BF, tag="zrep", name="zrep")
            nc.sync.dma_start(
                zrep[:], povs[b][64:65, None, hs, :].to_broadcast((1, 64, 4, QW)))
            nc.gpsimd.tensor_mul(on[b][g][0:64], povs[b][0:64, h0:h0 + 4:2],
                                 zrep[:, 0::2])
            ot_ = wpool.tile([64, 2, QW], BF, tag="otmp")
            nc.gpsimd.tensor_mul(ot_[:], povs[b][0:64, h0 + 1:h0 + 4:2],
                                 zrep[:, 1::2])
            nc.sync.dma_start(on[b][g][64:128], ot_[:])

        def tail1(b, h):
            # per-head low-latency variant for the final exposed group:
            # PE replicates the 1/Z row into PSUM, DVE applies it
            g, p, odd = h // 4, (h % 4) // 2, h % 2
            with nc.allow_low_precision(reason="Z scaled into f16 range; 2e-2 tol"):
                nc.vector.reciprocal(povs[b][64:65, h], povs[b][64:65, h])
            zr = ppO.tile([VW, QW], F32, tag="ov", name="psO")[0:64, :]
            nc.tensor.matmul(zr, ones_t[64:65, :], povs[b][64:65, h],
                             start=True, stop=True)
            if odd:
                o1 = wpool.tile([64, QW], BF, tag="otmp1")
                nc.vector.tensor_mul(o1[:], povs[b][0:64, h], zr)
                nc.sync.dma_start(on[b][g][64:128, p], o1[:])
            else:
                nc.vector.tensor_mul(on[b][g][0:64, p], povs[b][0:64, h], zr)

        def proj(b):
            outsb = wpool.tile([128, 6, QW], BF, tag="outsb")
            pw_s = pw_box[0]
            for c0 in (0, 3):
                pss = [ppS.tile([128, QW], F32, tag="s", name=f"psP{c0+i}")
                       for i in range(3)]
                for h in range(LH):
                    for i, ps in enumerate(pss):
                        nc.tensor.matmul(
                            ps, pw_s[:, h, (c0 + i) * 128:(c0 + i + 1) * 128],
                            on[b][:, h, :],
                            start=(h == 0), stop=(h == LH - 1))
                for i, ps in enumerate(pss):
                    nc.scalar.copy(outsb[:, c0 + i], ps)
            nc.sync.dma_start(
                out.ap()[b].rearrange("c p q -> p c q"), outsb[:])

        # ---- schedule ----
        # PE warmup: tiny matmuls so the p-state ramp completes before the
        # real projection chains arrive
        for _ in range(60):
            zw = ppZ.tile([128, QW], F32, tag="zr", name="psZ")[0:64, 0:64]
            nc.tensor.matmul(zw, ones_t[64:65, :], ones_t[64:65, :],
                             start=True, stop=True)

        qk_proj(0)
        scores(0)
        v_proj(0, 0)

        def run_head(b, h):
            head(b, h, mwq[h])
            if b == 1 and h >= LH - 4:
                tail1(b, h)
            elif h % 4 == 3:
                tail4(b, h - 3)

        run_head(0, 0)
        run_head(0, 1)
        qk_proj(1)
        scores(1)
        v_proj(1, 0)
        pss0 = [None]
        outsb_box = [None]

        def close5(b, pss, outsb):
            # final 4 head-steps on the 5 open chains, then per-chunk copies
            # and immediate DMAs so the output drains as it lands
            pw_s = pw_box[0]
            for pp in range(4, 6):
                for cc, ps in enumerate(pss):
                    nc.tensor.matmul(ps, pw_s[:, pp, cc * 128:(cc + 1) * 128],
                                     on[b][pp // 2][:, pp % 2, :],
                                     start=False, stop=(pp == 5))
            for cc, ps in enumerate(pss):
                if cc % 2 == 0:
                    nc.scalar.copy(outsb[:, cc], ps)
                else:
                    nc.vector.tensor_copy(outsb[:, cc], ps)
                nc.sync.dma_start(out.ap()[b, cc], outsb[:, cc])

        def chunk5(b, outsb):
            # full-width chunk-5 chain on the psZ bank (zr lives in ppO now)
            pw_s = pw_box[0]
            ps6 = ppZ.tile([128, QW], F32, tag="zr", name="psZ")
            for pp in range(6):
                nc.tensor.matmul(ps6, pw_s[:, pp, 640:768],
                                 on[b][pp // 2][:, pp % 2, :],
                                 start=(pp == 0), stop=(pp == 5))
            nc.vector.tensor_copy(outsb[:, 5], ps6)
            nc.sync.dma_start(out.ap()[b, 5], outsb[:, 5])

        def open_steps(pss, b, p0, p1):
            # pair-steps: contraction over 128 = 2 heads x 64 dims
            pw_s = pw_box[0]
            for pp in range(p0, p1):
                for cc, ps in enumerate(pss):
                    nc.tensor.matmul(ps, pw_s[:, pp, cc * 128:(cc + 1) * 128],
                                     on[b][pp // 2][:, pp % 2, :],
                                     start=(pp == 0), stop=False)

        for h in range(2, LH):
            run_head(0, h)
            run_head(1, h - 2)
            if h == 2:
                pw_s = cpool.tile([128, 6, C], BF, tag="pw")
                nc.sync.dma_start(pw_s[:, 0:3], pw.ap()[:, 0:3])
                pw_box[0] = pw_s
            if h == 3:
                v_proj(0, 1)
                nc.sync.dma_start(pw_box[0][:, 3:6], pw.ap()[:, 3:6])
            if h == 4:
                v_proj(1, 1)
            if h + 3 < LH:
                mwq[h + 3] = mw_load(h + 3)
            if h == 8:
                pss0[0] = [ppS.tile([128, QW], F32, tag="s", name=f"psP{i}")
                           for i in range(3)]
                pss0[0] += [ppA.tile([128, 512], F32, tag="bigA",
                                     name=f"psPA{i}")[:, :QW] for i in range(2)]
                open_steps(pss0[0], 0, 0, 2)
            if h == 9:
                open_steps(pss0[0], 0, 2, 4)
            if h == 11:
                outsb_box[0] = wpool.tile([128, 6, QW], BF, tag="outsb", name="outsb0")
                chunk5(0, outsb_box[0])
        run_head(1, LH - 1)
        close5(0, pss0[0], outsb_box[0])
        pss1 = [ppS.tile([128, QW], F32, tag="s", name=f"psQ{i}")
                for i in range(3)]
        pss1 += [ppA.tile([128, 512], F32, tag="bigA", name=f"psR{i}")[:, :QW]
                 for i in range(2)]
        open_steps(pss1, 1, 0, 5)
        ps6b1 = ppZ.tile([128, QW], F32, tag="zr", name="psZ")
        for pp in range(5):
            nc.tensor.matmul(ps6b1, pw_s[:, pp, 640:768],
                             on[1][pp // 2][:, pp % 2, :],
                             start=(pp == 0), stop=False)
        run_head(1, LH - 2)
        outsb1 = wpool.tile([128, 6, QW], BF, tag="outsb")
        for cc, ps in enumerate(pss1):
            nc.tensor.matmul(ps, pw_s[:, 5, cc * 128:(cc + 1) * 128],
                             on[1][2][:, 1, :], start=False, stop=True)
        nc.tensor.matmul(ps6b1, pw_s[:, 5, 640:768], on[1][2][:, 1, :],
                         start=False, stop=True)
        for cc, ps in enumerate(pss1 + [ps6b1]):
            if cc % 2 == 0:
                nc.scalar.copy(outsb1[:, cc], ps)
            else:
                nc.vector.tensor_copy(outsb1[:, cc], ps)
        nc.sync.dma_start(
            out.ap()[1, 0:3].rearrange("c p q -> p c q"), outsb1[:, 0:3])
        nc.sync.dma_start(
            out.ap()[1, 3:6].rearrange("c p q -> p c q"), outsb1[:, 3:6])

    nc.compile()
    return nc


def prep_inputs3(x, masks, Wq, Wk, Wv, mask_proj, proj_w, proj_b):
    """Build the 8 per-core input maps."""
    f16 = np.float16

    xhatT = np.zeros((B, C, NP), np.float32)
    xhatT[:, :, :N] = x.transpose(0, 2, 1)
    xta = np.ascontiguousarray(
        xhatT.reshape(B, KQ, 128, NP).transpose(0, 2, 1, 3)).astype(f16)

    def wpad(w, scale=1.0):
        return np.ascontiguousarray(
            (w * scale).reshape(KQ, 128, -1).transpose(1, 0, 2)).astype(f16)

    wqp = wpad(Wq, SCALE)
    wkp = wpad(Wk)

    # v weights interleaved per head as [v_h (64) | zero ones-col]
    wvh = np.zeros((C, LH * VW), np.float32)
    for h in range(LH):
        wvh[:, h * VW:h * VW + HD] = Wv[:, h * HD:(h + 1) * HD]
    wvp = wpad(wvh, 1.0 / 64.0)

    # token-validity mask -> the per-head ones columns of v-hat
    vmp = np.zeros((128, NJ, LH), np.float32)
    for j in range(NJ):
        lim = min(max(N - j * 128, 0), 128)
        vmp[:lim, j, :] = 1.0 / 64.0
    vmp = vmp.astype(f16)

    pwp = np.ascontiguousarray(
        proj_w.reshape(6, 2, 64, C).transpose(1, 2, 0, 3).reshape(128, 6, C)
    ).astype(f16)

    # host-folded mask weights: [k, q, g, h] zero-padded in k
    mw_nn = (masks.reshape(-1, ML).astype(np.float64)
             @ mask_proj.astype(np.float64)).astype(np.float32)
    mw_nn = mw_nn.reshape(N, N, GH, LH)          # [q, k, g, h]
    mw_kq = np.zeros((NP, N, GH, LH), np.float32)
    mw_kq[:N] = mw_nn.transpose(1, 0, 2, 3)      # [k, q, g, h]
    mw_full = np.ascontiguousarray(
        mw_kq.reshape(NJ, 128, N, GH, LH).transpose(1, 4, 3, 0, 2)).astype(f16)

    in_maps = []
    for c in range(8):
        p, s = c // 2, c % 2
        qo = 288 * s
        bsl = slice(2 * p, 2 * p + 2)
        in_maps.append({
            "xt": np.ascontiguousarray(xta[bsl].transpose(1, 0, 2, 3)),
            "xq": np.ascontiguousarray(
                xta[bsl, :, :, qo:qo + QW].transpose(1, 0, 2, 3)),
            "wq": wqp, "wk": wkp, "wv": wvp, "vm": vmp, "pw": pwp,
            "mw": np.ascontiguousarray(mw_full[:, :, :, :, qo:qo + QW]),
        })
    return in_maps


_NC3 = None


def get_nc3():
    global _NC3
    if _NC3 is None:
        _NC3 = build_nc3()
    return _NC3


def kernel(x, masks, Wq, Wk, Wv, mask_proj, proj_w, proj_b):
    x = np.asarray(x, np.float32)
    proj_b = np.asarray(proj_b, np.float32)
    in_maps = prep_inputs3(
        x, np.asarray(masks, np.float32), np.asarray(Wq, np.float32),
        np.asarray(Wk, np.float32), np.asarray(Wv, np.float32),
        np.asarray(mask_proj, np.float32), np.asarray(proj_w, np.float32),
        proj_b)
    res = bass_utils.run_bass_kernel_spmd(get_nc3(), in_maps, core_ids=list(range(8)))
    out = np.zeros((B, N, C), np.float32)
    for c in range(8):
        p, s = c // 2, c % 2
        ot = np.asarray(res.results[c]["ot"], np.float32)  # [2, 6, 128, QW]
        ot = ot.reshape(2, C, QW).transpose(0, 2, 1)       # [2, QW, C]
        for i, b in enumerate(range(2 * p, 2 * p + 2)):
            if s == 0:
                out[b, 0:289] = ot[i]
            else:
                out[b, 289:577] = ot[i, 1:]
    return (out + proj_b).astype(np.float32)

